# revision 21
# baseline (speedup 1.0000x reference)
"""Trainium2 Bass kernel for ChannelCompression:
   y = minmax_norm_spatial(leaky_relu(circulant_1x1_conv(x) + b))

Sharding: pure data parallel over batch (16 batches -> 2 per core x 8 cores).

Per-core strategy (memory-roofline bound: read x once, write y once):
  - View each batch as [C=16, G=8, S=32768] and stack (c,g) onto the 128
    SBUF partitions.  The circulant 16x16 conv becomes one 128x128
    block-structured matmul weight kron(W2.T, I8), so every PE column
    computes all 16 output channels for 8 spatial groups at once.
  - All DMA moves in 2 MiB chunks (4096 f32 cols): 1 MiB transfers only
    sustain ~340 GB/s of the ~360 GB/s HBM/NC budget; 2 MiB ~380+.
    Output goes through HWDGE (full rate) from f32 staging tiles — the
    SWDGE bf16->f32 cast-DMA path measured only ~50% engine duty.
  - Pass 1 streams x chunks in, matmuls into PSUM, applies leaky-relu
    (+bias) on ScalarE writing two BF16 resident y tiles per chunk
    (bf16 precision is amply covered by the 2e-2 gate).
  - Stats: per chunk, DVE tensor_tensor min/max of the two 2048-col
    tiles (bf16 2x DVE mode) + one [128,2,1024]->2 reduce per stat;
    this keeps DVE under the input-DMA rate so the fold never lags.
  - Per-batch stats fold: tiny PE transposes into free-dim space,
    reduce over the 8 spatial groups, invert, broadcast back via two
    selector matmuls -> per-partition scale/bias [128,2].
  - Pass 2 normalizes y (GpSimd, f32 math) into f32 staging chunks and
    streams them out via HWDGE.  24 ypool bufs give batch b+1's pass 1
    eight chunks of runway before it couples to batch b's pass 2.
"""

import numpy as np
from contextlib import ExitStack

import concourse.bacc as bacc
import concourse.tile as tile
import concourse.bass as bass
from concourse import mybir
from concourse.bass_utils import run_bass_kernel_spmd

F32 = mybir.dt.float32
BF16 = mybir.dt.bfloat16
AF = mybir.ActivationFunctionType
ALU = mybir.AluOpType
AX = mybir.AxisListType

N_CORES = 8
B, C, H, W = 16, 16, 512, 512
G = 8                   # spatial groups stacked into partitions
BP = B // N_CORES       # batches per core
S_FULL = (H * W) // G   # 32768 spatial elems per group
CK = 4096               # columns per DMA chunk (2 MiB fp32 HBM-side)
TS = 2048               # columns per resident y tile (2 per chunk)
PT = 1024               # columns per PSUM tile (2 banks)
MM = 512                # columns per matmul (1 PSUM bank, fp32 moving max)
EPS = 1e-8
NEG_SLOPE = 0.1


def build_nc(S=S_FULL, ck=CK):
    n_c = S // ck       # chunks per batch (8)
    n_t = S // TS       # y tiles per batch (16)
    nc = bacc.Bacc("TRN2", target_bir_lowering=False)

    xs = nc.dram_tensor("x", [BP, C, G, S], F32, kind="ExternalInput")
    wbd = nc.dram_tensor("wbd", [128, 128], F32, kind="ExternalInput")
    ident = nc.dram_tensor("ident", [128, 128], F32, kind="ExternalInput")
    identb = nc.dram_tensor("identb", [128, 128], BF16, kind="ExternalInput")
    sel = nc.dram_tensor("sel", [32, 2, 128], F32, kind="ExternalInput")
    bb = nc.dram_tensor("b128", [128, 1], F32, kind="ExternalInput")
    ys = nc.dram_tensor("y", [BP, C, G, S], F32, kind="ExternalOutput")

    with tile.TileContext(nc) as tc, ExitStack() as ctx:
        consts = ctx.enter_context(tc.tile_pool(name="consts", bufs=1))
        xpool = ctx.enter_context(tc.tile_pool(name="xpool", bufs=3))
        ypool = ctx.enter_context(tc.tile_pool(name="ypool", bufs=n_t + 8))
        opool = ctx.enter_context(tc.tile_pool(name="opool", bufs=2))
        ppool = ctx.enter_context(tc.tile_pool(name="ppool", bufs=2))
        spool = ctx.enter_context(tc.tile_pool(name="stats", bufs=2))
        small = ctx.enter_context(tc.tile_pool(name="small", bufs=2))
        psum = ctx.enter_context(tc.tile_pool(name="psum", bufs=2, space="PSUM"))
        psmall = ctx.enter_context(tc.tile_pool(name="psmall", bufs=2, space="PSUM"))

        wbd_sb = consts.tile([128, 128], F32)
        nc.gpsimd.dma_start(out=wbd_sb, in_=wbd[:])
        id_sb = consts.tile([128, 128], F32)
        nc.gpsimd.dma_start(out=id_sb, in_=ident[:])
        idb_sb = consts.tile([128, 128], BF16)
        nc.gpsimd.dma_start(out=idb_sb, in_=identb[:])
        sel_sb = consts.tile([32, 2, 128], F32)
        nc.gpsimd.dma_start(out=sel_sb, in_=sel[:])
        b_sb = consts.tile([128, 1], F32)
        nc.gpsimd.dma_start(out=b_sb, in_=bb[:])

        state = {}

        def pass1_chunk(bi, c):
            """DMA in x chunk c of batch bi, conv+lrelu into resident y, stats."""
            st_min, st_max, y_tiles = state[bi][:3]
            xt = xpool.tile([128, ck], F32, tag="x")
            nc.sync.dma_start(out=xt, in_=xs[bi, :, :, c * ck:(c + 1) * ck])
            ya = ypool.tile([128, TS], BF16, tag="y")
            yb = ypool.tile([128, TS], BF16, tag="y")
            for j in range(ck // PT):
                pt = psum.tile([128, PT], F32, tag="ps")
                for k in range(PT // MM):
                    c0 = k * MM
                    nc.tensor.matmul(
                        pt[:, c0:c0 + MM],
                        wbd_sb,
                        xt[:, j * PT + c0:j * PT + c0 + MM],
                        start=True,
                        stop=True,
                    )
                # y = leaky_relu(conv + b): fused PSUM->SBUF (bf16) on ScalarE
                yh = ya if j < 2 else yb
                nc.scalar.activation(
                    out=yh[:, (j % 2) * PT:(j % 2 + 1) * PT],
                    in_=pt,
                    func=AF.Prelu,
                    bias=b_sb,
                    scale=1.0,
                    alpha=NEG_SLOPE,
                )
            y_tiles.append(ya)
            y_tiles.append(yb)
            # stats: bf16 tensor_tensor of the chunk's tiles (2x DVE mode),
            # then one reduce per stat with a 2-elem output
            pmn = ppool.tile([128, TS], BF16, tag="pmn")
            nc.vector.tensor_tensor(out=pmn, in0=ya, in1=yb, op=ALU.min)
            pmx = ppool.tile([128, TS], BF16, tag="pmx")
            nc.vector.tensor_tensor(out=pmx, in0=ya, in1=yb, op=ALU.max)
            nc.vector.tensor_reduce(
                out=st_min[:, 2 * c:2 * c + 2],
                in_=pmn.rearrange("p (r q) -> p r q", r=2),
                axis=AX.X,
                op=ALU.min,
            )
            nc.vector.tensor_reduce(
                out=st_max[:, 2 * c:2 * c + 2],
                in_=pmx.rearrange("p (r q) -> p r q", r=2),
                axis=AX.X,
                op=ALU.max,
            )

        def stats_fold(bi):
            """Fold per-partition stats into per-partition scale/bias [128,2]."""
            st_min, st_max = state[bi][:2]
            s2 = small.tile([128, 2], BF16, tag="s2")
            nc.vector.tensor_reduce(out=s2[:, 0:1], in_=st_min, axis=AX.X, op=ALU.min)
            nc.vector.tensor_reduce(out=s2[:, 1:2], in_=st_max, axis=AX.X, op=ALU.max)
            # transpose [128,1] stats into free dim (partition 0)
            ptr = psmall.tile([1, 256], BF16, tag="psmb")
            nc.tensor.transpose(ptr[:, 0:128], s2[:, 0:1], idb_sb)
            nc.tensor.transpose(ptr[:, 128:256], s2[:, 1:2], idb_sb)
            tl = small.tile([1, 256], F32, tag="tl")
            nc.scalar.copy(out=tl[:, 0:128], in_=ptr[:, 0:128])
            nc.scalar.copy(out=tl[:, 128:256], in_=ptr[:, 128:256])
            # reduce over the 8 groups (free index p = o*8+g)
            u = small.tile([1, 32], F32, tag="u")
            nc.vector.tensor_reduce(
                out=u[:, 0:16],
                in_=tl[:, 0:128].rearrange("p (o g) -> p o g", g=G),
                axis=AX.X,
                op=ALU.min,
            )
            nc.vector.tensor_reduce(
                out=u[:, 16:32],
                in_=tl[:, 128:256].rearrange("p (o g) -> p o g", g=G),
                axis=AX.X,
                op=ALU.max,
            )
            # scale = 1/(mx-mn+eps); nbias = -mn*scale
            v = small.tile([1, 16], F32, tag="v")
            nc.vector.tensor_sub(out=v, in0=u[:, 16:32], in1=u[:, 0:16])
            vv = small.tile([1, 16], F32, tag="vv")
            nc.vector.tensor_scalar(
                out=vv, in0=v, scalar1=EPS, scalar2=None, op0=ALU.add
            )
            pk = small.tile([1, 32], F32, tag="pk")
            nc.vector.reciprocal(out=pk[:, 0:16], in_=vv)
            tmp = small.tile([1, 16], F32, tag="tmp")
            nc.vector.tensor_mul(out=tmp, in0=u[:, 0:16], in1=pk[:, 0:16])
            nc.vector.tensor_scalar(
                out=pk[:, 16:32], in0=tmp, scalar1=-1.0, scalar2=None, op0=ALU.mult
            )
            # broadcast [1,32] free-dim -> per-partition [128,2] via transpose
            # + selector matmuls (sel[k,0,p]=d(k==p//8), sel[k,1,p]=d(k-16==p//8))
            pz = psmall.tile([32, 1], F32, tag="psm")
            nc.tensor.transpose(pz, pk, id_sb[0:1, 0:1])
            zs = small.tile([32, 1], F32, tag="zs")
            nc.scalar.copy(out=zs, in_=pz)
            pb1 = psmall.tile([128, 1], F32, tag="psm")
            nc.tensor.matmul(pb1, sel_sb[:, 0, :], zs, start=True, stop=True)
            pb2 = psmall.tile([128, 1], F32, tag="psm")
            nc.tensor.matmul(pb2, sel_sb[:, 1, :], zs, start=True, stop=True)
            sc = small.tile([128, 2], F32, tag="sc")
            nc.scalar.copy(out=sc[:, 0:1], in_=pb1)
            nc.scalar.copy(out=sc[:, 1:2], in_=pb2)
            return sc

        def pass2_chunk(bi, c, sc):
            """Normalize two resident bf16 y tiles (GpSimd, f32 math) into an
            f32 staging chunk and stream it out via HWDGE."""
            y_tiles = state[bi][2]
            ot = opool.tile([128, ck], F32, tag="o")
            for h in range(2):
                nc.gpsimd.tensor_scalar(
                    out=ot[:, h * TS:(h + 1) * TS],
                    in0=y_tiles[2 * c + h],
                    scalar1=sc[:, 0:1],
                    scalar2=sc[:, 1:2],
                    op0=ALU.mult,
                    op1=ALU.add,
                )
            nc.scalar.dma_start(out=ys[bi, :, :, c * ck:(c + 1) * ck], in_=ot)

        # software pipeline: pass1(0); then per batch: pre-emit the first
        # next-batch chunks (keeps PE busy through the stats fold), fold
        # stats, then interleave pass2(bi) with the rest of pass1(bi+1).
        PRE = 3
        for bi in range(BP):
            state[bi] = (
                spool.tile([128, 2 * n_c], BF16, tag="stmin", name=f"stmin{bi}"),
                spool.tile([128, 2 * n_c], BF16, tag="stmax", name=f"stmax{bi}"),
                [],
                None,
            )
        for c in range(n_c):
            pass1_chunk(0, c)
        for bi in range(BP):
            if bi + 1 < BP:
                for c in range(PRE):
                    pass1_chunk(bi + 1, c)
            sc = stats_fold(bi)
            for c in range(n_c):
                pass2_chunk(bi, c, sc)
                if bi + 1 < BP and c + PRE < n_c:
                    pass1_chunk(bi + 1, c + PRE)

    nc.compile()
    return nc


def host_consts(w, b):
    """Host-side tiny constant tensors fed to every core."""
    import ml_dtypes
    w = np.asarray(w, np.float32).reshape(16)
    b = np.asarray(b, np.float32).reshape(1)
    W2 = np.stack([np.roll(w, o) for o in range(16)], axis=0)   # [O,C]
    wbd = np.kron(W2.T.copy(), np.eye(G, dtype=np.float32))     # [128,128]
    wbd = np.ascontiguousarray(wbd, np.float32)
    ident = np.eye(128, dtype=np.float32)
    identb = np.eye(128, dtype=np.float32).astype(ml_dtypes.bfloat16)
    sel = np.zeros((32, 2, 128), np.float32)
    for p in range(128):
        sel[p // G, 0, p] = 1.0
        sel[16 + p // G, 1, p] = 1.0
    b128 = np.full((128, 1), float(b[0]), np.float32)
    return wbd, ident, identb, sel, b128


_NC = None
LAST_RESULTS = None


def kernel(x, w, b):
    global _NC, LAST_RESULTS
    x = np.ascontiguousarray(np.asarray(x, np.float32))
    assert x.shape == (B, C, H, W)
    if _NC is None:
        _NC = build_nc()
    wbd, ident, identb, sel, b128 = host_consts(w, b)

    xg = x.reshape(N_CORES, BP, C, G, S_FULL)
    in_maps = [
        {
            "x": np.ascontiguousarray(xg[ci]),
            "wbd": wbd,
            "ident": ident,
            "identb": identb,
            "sel": sel,
            "b128": b128,
        }
        for ci in range(N_CORES)
    ]
    res = run_bass_kernel_spmd(_NC, in_maps, core_ids=list(range(N_CORES)))
    LAST_RESULTS = res
    out = np.concatenate([r["y"].reshape(BP, C, H, W) for r in res.results], axis=0)
    return out


# revision 22
# speedup vs baseline: 1.1427x; 1.1427x over previous
"""Trainium2 Bass kernel for ChannelCompression:
   y = minmax_norm_spatial(leaky_relu(circulant_1x1_conv(x) + b))

Sharding: pure data parallel over batch (16 batches -> 2 per core x 8 cores).

Per-core strategy (memory-roofline bound: read x once, write y once):
  - View each batch as [C=16, G=8, S=32768] and stack (c,g) onto the 128
    SBUF partitions.  The circulant 16x16 conv becomes one 128x128
    block-structured matmul weight kron(W2.T, I8), so every PE column
    computes all 16 output channels for 8 spatial groups at once.
  - DMA moves in 2 MiB chunks (4096 cols): 1 MiB transfers only sustain
    ~340 GB/s of the ~360 GB/s HBM/NC budget; 2-4 MiB gets ~390+.
  - Pass 1 streams x chunks in, matmuls into PSUM, applies leaky-relu
    (+bias) on ScalarE writing a monolithic BF16 resident y per batch
    (8 MiB; bf16 precision amply covered by the 2e-2 gate).
  - Stats: per chunk, DVE tensor_tensor min/max of the two 2048-col
    halves (bf16 2x DVE mode) + one [128,2,1024]->2 reduce per stat.
  - Per-batch stats fold: tiny PE transposes into free-dim space,
    reduce over the 8 spatial groups, invert, broadcast back via two
    selector matmuls -> per-partition scale/bias [128,2].
  - Pass 2 normalizes y in place per 4096-col chunk (alternating GpSimd
    / DVE so neither engine paces the tail), then streams out via SWDGE
    cast-DMA (bf16 SBUF -> f32 HBM), no f32 staging pass.
  - Batches are fully decoupled: batch 1's pass 1 never waits on batch
    0's pass 2, so the DMA queues never drain.
"""

import numpy as np
from contextlib import ExitStack

import concourse.bacc as bacc
import concourse.tile as tile
import concourse.bass as bass
from concourse import mybir
from concourse.bass_utils import run_bass_kernel_spmd

F32 = mybir.dt.float32
BF16 = mybir.dt.bfloat16
AF = mybir.ActivationFunctionType
ALU = mybir.AluOpType
AX = mybir.AxisListType

N_CORES = 8
B, C, H, W = 16, 16, 512, 512
G = 8                   # spatial groups stacked into partitions
BP = B // N_CORES       # batches per core
S_FULL = (H * W) // G   # 32768 spatial elems per group
CK = 4096               # columns per DMA chunk (2 MiB fp32 HBM-side)
PT = 1024               # columns per PSUM tile (2 banks)
MM = 512                # columns per matmul (1 PSUM bank, fp32 moving max)
EPS = 1e-8
NEG_SLOPE = 0.1


def build_nc(S=S_FULL, ck=CK):
    n_c = S // ck       # chunks per batch (8)
    nc = bacc.Bacc("TRN2", target_bir_lowering=False)

    xs = nc.dram_tensor("x", [BP, C, G, S], F32, kind="ExternalInput")
    wbd = nc.dram_tensor("wbd", [128, 128], F32, kind="ExternalInput")
    ident = nc.dram_tensor("ident", [128, 128], F32, kind="ExternalInput")
    identb = nc.dram_tensor("identb", [128, 128], BF16, kind="ExternalInput")
    sel = nc.dram_tensor("sel", [32, 2, 128], F32, kind="ExternalInput")
    bb = nc.dram_tensor("b128", [128, 1], F32, kind="ExternalInput")
    ys = nc.dram_tensor("y", [BP, C, G, S], F32, kind="ExternalOutput")

    with tile.TileContext(nc) as tc, ExitStack() as ctx:
        consts = ctx.enter_context(tc.tile_pool(name="consts", bufs=1))
        xpool = ctx.enter_context(tc.tile_pool(name="xpool", bufs=4))
        ypool = ctx.enter_context(tc.tile_pool(name="ypool", bufs=1))
        ppool = ctx.enter_context(tc.tile_pool(name="ppool", bufs=1))
        spool = ctx.enter_context(tc.tile_pool(name="stats", bufs=2))
        small = ctx.enter_context(tc.tile_pool(name="small", bufs=2))
        psum = ctx.enter_context(tc.tile_pool(name="psum", bufs=2, space="PSUM"))
        psmall = ctx.enter_context(tc.tile_pool(name="psmall", bufs=2, space="PSUM"))

        wbd_sb = consts.tile([128, 128], F32)
        nc.gpsimd.dma_start(out=wbd_sb, in_=wbd[:])
        id_sb = consts.tile([128, 128], F32)
        nc.gpsimd.dma_start(out=id_sb, in_=ident[:])
        idb_sb = consts.tile([128, 128], BF16)
        nc.gpsimd.dma_start(out=idb_sb, in_=identb[:])
        sel_sb = consts.tile([32, 2, 128], F32)
        nc.gpsimd.dma_start(out=sel_sb, in_=sel[:])
        b_sb = consts.tile([128, 1], F32)
        nc.gpsimd.dma_start(out=b_sb, in_=bb[:])

        state = {}

        def pass1_chunk(bi, c):
            """DMA in x chunk c of batch bi, conv+lrelu into resident y, stats."""
            st_min, st_max, y_full = state[bi][:3]
            xt = xpool.tile([128, ck], F32, tag="x")
            nc.sync.dma_start(out=xt, in_=xs[bi, :, :, c * ck:(c + 1) * ck])
            for j in range(ck // PT):
                pt = psum.tile([128, PT], F32, tag="ps")
                for k in range(PT // MM):
                    c0 = k * MM
                    nc.tensor.matmul(
                        pt[:, c0:c0 + MM],
                        wbd_sb,
                        xt[:, j * PT + c0:j * PT + c0 + MM],
                        start=True,
                        stop=True,
                    )
                # y = leaky_relu(conv + b): fused PSUM->SBUF (bf16) on ScalarE
                nc.scalar.activation(
                    out=y_full[:, c * ck + j * PT:c * ck + (j + 1) * PT],
                    in_=pt,
                    func=AF.Prelu,
                    bias=b_sb,
                    scale=1.0,
                    alpha=NEG_SLOPE,
                )
            # stats: bf16 tensor_tensor of the chunk halves (2x DVE mode),
            # then one reduce per stat with a 2-elem output
            h = ck // 2
            ya = y_full[:, c * ck:c * ck + h]
            yb = y_full[:, c * ck + h:(c + 1) * ck]
            pmn = ppool.tile([128, h], BF16, tag="pmn")
            nc.vector.tensor_tensor(out=pmn, in0=ya, in1=yb, op=ALU.min)
            pmx = ppool.tile([128, h], BF16, tag="pmx")
            nc.vector.tensor_tensor(out=pmx, in0=ya, in1=yb, op=ALU.max)
            nc.vector.tensor_reduce(
                out=st_min[:, 2 * c:2 * c + 2],
                in_=pmn.rearrange("p (r q) -> p r q", r=2),
                axis=AX.X,
                op=ALU.min,
            )
            nc.vector.tensor_reduce(
                out=st_max[:, 2 * c:2 * c + 2],
                in_=pmx.rearrange("p (r q) -> p r q", r=2),
                axis=AX.X,
                op=ALU.max,
            )

        def stats_fold(bi):
            """Fold per-partition stats into per-partition scale/bias [128,2]."""
            st_min, st_max = state[bi][:2]
            s2 = small.tile([128, 2], BF16, tag="s2")
            nc.vector.tensor_reduce(out=s2[:, 0:1], in_=st_min, axis=AX.X, op=ALU.min)
            nc.vector.tensor_reduce(out=s2[:, 1:2], in_=st_max, axis=AX.X, op=ALU.max)
            # transpose [128,1] stats into free dim (partition 0)
            ptr = psmall.tile([1, 256], BF16, tag="psmb")
            nc.tensor.transpose(ptr[:, 0:128], s2[:, 0:1], idb_sb)
            nc.tensor.transpose(ptr[:, 128:256], s2[:, 1:2], idb_sb)
            tl = small.tile([1, 256], F32, tag="tl")
            nc.scalar.copy(out=tl[:, 0:128], in_=ptr[:, 0:128])
            nc.scalar.copy(out=tl[:, 128:256], in_=ptr[:, 128:256])
            # reduce over the 8 groups (free index p = o*8+g)
            u = small.tile([1, 32], F32, tag="u")
            nc.vector.tensor_reduce(
                out=u[:, 0:16],
                in_=tl[:, 0:128].rearrange("p (o g) -> p o g", g=G),
                axis=AX.X,
                op=ALU.min,
            )
            nc.vector.tensor_reduce(
                out=u[:, 16:32],
                in_=tl[:, 128:256].rearrange("p (o g) -> p o g", g=G),
                axis=AX.X,
                op=ALU.max,
            )
            # scale = 1/(mx-mn+eps); nbias = -mn*scale
            v = small.tile([1, 16], F32, tag="v")
            nc.vector.tensor_sub(out=v, in0=u[:, 16:32], in1=u[:, 0:16])
            vv = small.tile([1, 16], F32, tag="vv")
            nc.vector.tensor_scalar(
                out=vv, in0=v, scalar1=EPS, scalar2=None, op0=ALU.add
            )
            pk = small.tile([1, 32], F32, tag="pk")
            nc.vector.reciprocal(out=pk[:, 0:16], in_=vv)
            tmp = small.tile([1, 16], F32, tag="tmp")
            nc.vector.tensor_mul(out=tmp, in0=u[:, 0:16], in1=pk[:, 0:16])
            nc.vector.tensor_scalar(
                out=pk[:, 16:32], in0=tmp, scalar1=-1.0, scalar2=None, op0=ALU.mult
            )
            # broadcast [1,32] free-dim -> per-partition [128,2] via transpose
            # + selector matmuls (sel[k,0,p]=d(k==p//8), sel[k,1,p]=d(k-16==p//8))
            pz = psmall.tile([32, 1], F32, tag="psm")
            nc.tensor.transpose(pz, pk, id_sb[0:1, 0:1])
            zs = small.tile([32, 1], F32, tag="zs")
            nc.scalar.copy(out=zs, in_=pz)
            pb1 = psmall.tile([128, 1], F32, tag="psm")
            nc.tensor.matmul(pb1, sel_sb[:, 0, :], zs, start=True, stop=True)
            pb2 = psmall.tile([128, 1], F32, tag="psm")
            nc.tensor.matmul(pb2, sel_sb[:, 1, :], zs, start=True, stop=True)
            sc = small.tile([128, 2], F32, tag="sc")
            nc.scalar.copy(out=sc[:, 0:1], in_=pb1)
            nc.scalar.copy(out=sc[:, 1:2], in_=pb2)
            return sc

        def pass2_chunk(bi, c, sc):
            """Normalize resident bf16 y chunk in place (f32 scalar math,
            GpSimd/DVE alternating) and stream out via SWDGE cast-DMA."""
            y_full = state[bi][2]
            sl = y_full[:, c * ck:(c + 1) * ck]
            eng = nc.gpsimd if c % 2 == 0 else nc.vector
            eng.tensor_scalar(
                out=sl,
                in0=sl,
                scalar1=sc[:, 0:1],
                scalar2=sc[:, 1:2],
                op0=ALU.mult,
                op1=ALU.add,
            )
            nc.gpsimd.dma_start(out=ys[bi, :, :, c * ck:(c + 1) * ck], in_=sl)

        # software pipeline: pass1(0); then per batch: pre-emit the first
        # next-batch chunks (keeps PE busy through the stats fold), fold
        # stats, then interleave pass2(bi) with the rest of pass1(bi+1).
        PRE = 3
        for bi in range(BP):
            state[bi] = (
                spool.tile([128, 2 * n_c], BF16, tag="stmin", name=f"stmin{bi}"),
                spool.tile([128, 2 * n_c], BF16, tag="stmax", name=f"stmax{bi}"),
                ypool.tile([128, S], BF16, tag=f"y{bi}", name=f"y{bi}"),
                None,
            )
        for c in range(n_c):
            pass1_chunk(0, c)
        for bi in range(BP):
            if bi + 1 < BP:
                for c in range(PRE):
                    pass1_chunk(bi + 1, c)
            sc = stats_fold(bi)
            for c in range(n_c):
                pass2_chunk(bi, c, sc)
                if bi + 1 < BP and c + PRE < n_c:
                    pass1_chunk(bi + 1, c + PRE)

    nc.compile()
    return nc


def host_consts(w, b):
    """Host-side tiny constant tensors fed to every core."""
    import ml_dtypes
    w = np.asarray(w, np.float32).reshape(16)
    b = np.asarray(b, np.float32).reshape(1)
    W2 = np.stack([np.roll(w, o) for o in range(16)], axis=0)   # [O,C]
    wbd = np.kron(W2.T.copy(), np.eye(G, dtype=np.float32))     # [128,128]
    wbd = np.ascontiguousarray(wbd, np.float32)
    ident = np.eye(128, dtype=np.float32)
    identb = np.eye(128, dtype=np.float32).astype(ml_dtypes.bfloat16)
    sel = np.zeros((32, 2, 128), np.float32)
    for p in range(128):
        sel[p // G, 0, p] = 1.0
        sel[16 + p // G, 1, p] = 1.0
    b128 = np.full((128, 1), float(b[0]), np.float32)
    return wbd, ident, identb, sel, b128


_NC = None
LAST_RESULTS = None


def kernel(x, w, b):
    global _NC, LAST_RESULTS
    x = np.ascontiguousarray(np.asarray(x, np.float32))
    assert x.shape == (B, C, H, W)
    if _NC is None:
        _NC = build_nc()
    wbd, ident, identb, sel, b128 = host_consts(w, b)

    xg = x.reshape(N_CORES, BP, C, G, S_FULL)
    in_maps = [
        {
            "x": np.ascontiguousarray(xg[ci]),
            "wbd": wbd,
            "ident": ident,
            "identb": identb,
            "sel": sel,
            "b128": b128,
        }
        for ci in range(N_CORES)
    ]
    res = run_bass_kernel_spmd(_NC, in_maps, core_ids=list(range(N_CORES)))
    LAST_RESULTS = res
    out = np.concatenate([r["y"].reshape(BP, C, H, W) for r in res.results], axis=0)
    return out
